# revision 2
# baseline (speedup 1.0000x reference)
"""FP16-pulse -> FP8(E4M3)-pulse converter, Trainium2 Bass/Tile kernel, v4.

v4 replaces v3's arithmetic rounding pipeline with the DVE's native
f16 -> f8e4 convert (verified exact vs the reference for every magnitude
pattern em <= 23551: RNE, subnormal outputs, and carry promote all match):

  em  = |fp16| bit pattern, assembled from the 15 magnitude pulse bits
        (7-lane pair op + base-4 tree, f32 lanes above 2048)
  em' = min(em, 23551)           # e>=23 handled by the +6 post-fix
  vham: Act writes em' to int16, bitcast f16 = |v| (exact)
  f8  = copy(|v| -> float8e4)    # the whole RNE/subnormal/saturate logic
  cb  = copy(bitcast u8 -> f16)  # 7-bit output code E*8+M
  c2  = cb + 6*(em > 23551)      # reference saturates every e>=23 to
                                 # (E,M)=(15,6); clamped input gives 120
  7 bits of c2 by is_ge/add chains; Act rescales each plane to 0/1 into
  the strided output planes; sign plane is a copy of input bit 0.

Transport: pulses cross HBM as float16 both ways (exact for 0/1; host only
casts dtype and slices -- all bit-level compute is on device).

Engine split: DVE runs the assembly STTs, the convert copies and the first
extraction level; Pool runs the remaining extraction levels and the em
clamp/overflow tensor_scalars; Act runs the int16 writeback and all eight
output planes. Loads and stores ride the SP HWDGE queue with lookahead.
"""

import numpy as np
from contextlib import ExitStack

import concourse.bass as bass
import concourse.bacc as bacc
import concourse.tile as tile
from concourse import mybir
from concourse.bass_utils import run_bass_kernel_spmd

F32 = mybir.dt.float32
F16 = mybir.dt.float16
I16 = mybir.dt.int16
F8 = mybir.dt.float8e4
U8 = mybir.dt.uint8
OP = mybir.AluOpType
ACTF = mybir.ActivationFunctionType

P = 128
N_CORES = 8
B0, B1 = 4096, 4096
NBITS, OBITS = 16, 8

ROWS = B0 // N_CORES                    # 512 rows per core
VALS = ROWS * B1 // P                   # 16384 values per partition

# kept for test.py compatibility
VPT_FULL = 512
NTILES_FULL = VALS // VPT_FULL


def tile_sizes(mid=640, head=(256, 256), tail=(512, 384, 256)):
    head, tail = list(head), list(tail)
    n = (VALS - sum(head) - sum(tail)) // mid
    assert sum(head) + sum(tail) + n * mid == VALS
    return head + [mid] * n + tail


def build_nc(lookahead: int = 3, xbufs: int = 3, mid: int = 640,
             ext_split: int = 1, clamp_on_pool: bool = True,
             ext_plan: str | None = "VVVVGG", f8_on_pool: bool = True,
             cb_on_pool: bool = True, c2_on_pool: bool = False) -> bass.Bass:
    # ext_plan: 6 chars from {'V','G','M'} per level: V=both DVE, G=both
    # Pool, M=mixed (TS on DVE, TT on Pool). Overrides ext_split.
    nc = bacc.Bacc()
    x = nc.declare_dram_parameter("x", [P, VALS * NBITS], F16, isOutput=False)
    y = nc.declare_dram_parameter("y", [P, VALS * OBITS], F16, isOutput=True)

    sizes = tile_sizes(mid=mid)

    with tile.TileContext(nc) as tc, ExitStack() as ctx:
        iop = ctx.enter_context(tc.tile_pool(name="io", bufs=2))
        tp = ctx.enter_context(tc.tile_pool(name="tmp", bufs=2))
        V, G, S, SP = nc.vector, nc.gpsimd, nc.scalar, nc.sync

        offs = np.cumsum([0] + sizes).tolist()
        xts = {}

        def emit_load(i):
            t = sizes[i]
            xt = iop.tile([P, NBITS * t], F16, tag="x", name="xt", bufs=xbufs)
            SP.dma_start(xt[:], x[:, offs[i] * NBITS:(offs[i] + t) * NBITS])
            xts[i] = xt

        for k in range(min(lookahead, len(sizes))):
            emit_load(k)
        for i, tsz in enumerate(sizes):
            xt = xts.pop(i)
            off = offs[i]
            xb = xt[:].rearrange("p (v b) -> p v b", b=NBITS)
            yt = iop.tile([P, OBITS * tsz], F16, tag="y", name="yt", bufs=3)
            yb = yt[:].rearrange("p (v b) -> p v b", b=OBITS)

            def vt(tag, dt=F16, w=1):
                return tp.tile([P, tsz * w], dt, tag=tag, name=tag)

            # ---- DVE: em assembly (15 magnitude bits -> |v| pattern) ---
            # L1: 7 bit pairs p_j = 2*b(1+2j) + b(2+2j), j=0..6 (bits 1..14)
            pc = vt("pc", w=7)
            pcb = pc[:].rearrange("p (v k) -> p v k", k=7)
            V.scalar_tensor_tensor(pcb[:, :, :], xb[:, :, 1:14:2], 2.0,
                                   xb[:, :, 2:15:2], OP.mult, OP.add)
            # L2: q_j = 4*p(2j) + p(2j+1), j=0..2 (pairs of pairs)
            q3 = vt("q3", w=3)
            q3b = q3[:].rearrange("p (v k) -> p v k", k=3)
            V.scalar_tensor_tensor(q3b[:, :, :], pcb[:, :, 0:5:2], 4.0,
                                   pcb[:, :, 1:6:2], OP.mult, OP.add)
            # L3/L4: base-16 Horner over q0..q2, then p6 and b15
            r0 = vt("r0")                       # <= 255, f16 exact
            V.scalar_tensor_tensor(r0[:], q3b[:, :, 0], 16.0, q3b[:, :, 1],
                                   OP.mult, OP.add)
            r1 = vt("r1", F32)                  # <= 4095
            V.scalar_tensor_tensor(r1[:], r0[:], 16.0, q3b[:, :, 2],
                                   OP.mult, OP.add)
            H = vt("H", F32)                    # <= 16383
            V.scalar_tensor_tensor(H[:], r1[:], 4.0, pcb[:, :, 6],
                                   OP.mult, OP.add)
            em = vt("em", F32)                  # <= 32767
            V.scalar_tensor_tensor(em[:], H[:], 2.0, xb[:, :, 15],
                                   OP.mult, OP.add)

            # ---- clamp + overflow fix-up -------------------------------
            ceng = G if clamp_on_pool else V
            emc = vt("emc", F32)
            ceng.tensor_scalar(emc[:], em[:], 23551.0, None, OP.min)
            ovf6 = vt("ovf6")
            ceng.tensor_scalar(ovf6[:], em[:], 23551.5, 6.0, OP.is_gt, OP.mult)

            # ---- Act int16 writeback, DVE f8 convert -------------------
            vham = vt("vham", I16)
            S.activation(vham[:], emc[:], ACTF.Copy, bias=0.0, scale=1.0)
            f8 = vt("f8", F8)
            (G if f8_on_pool else V).tensor_copy(f8[:], vham[:].bitcast(F16))
            cb = vt("cb")
            (G if cb_on_pool else V).tensor_copy(cb[:], f8[:].bitcast(U8))
            c2 = vt("c2")
            (G if c2_on_pool else V).tensor_tensor(c2[:], cb[:], ovf6[:], OP.add)

            # ---- bit extraction: 6 levels, split DVE/Pool --------------
            srcs = []   # (plane_idx, tile, scale)
            r = c2
            for lvl in range(6):
                k = 6 - lvl          # bit index being extracted
                if ext_plan:
                    ts_eng = V if ext_plan[lvl] in "VM" else G
                    tt_eng = V if ext_plan[lvl] == "V" else G
                else:
                    ts_eng = tt_eng = V if lvl < ext_split else G
                bs = vt(f"b{k}s")
                ts_eng.tensor_scalar(bs[:], r[:], float(1 << k),
                                     -float(1 << k), OP.is_ge, OP.mult)
                rn = vt(f"rr{k - 1}")
                tt_eng.tensor_tensor(rn[:], r[:], bs[:], OP.add)
                srcs.append((7 - k, bs, -1.0 / (1 << k)))
                r = rn
            srcs.append((7, r, 1.0))  # bit 0, already 0/1

            # ---- Act: output planes ------------------------------------
            S.activation(yb[:, :, 0], xb[:, :, 0], ACTF.Copy,
                         bias=0.0, scale=1.0)
            for pj, src, sc in srcs:
                S.activation(yb[:, :, pj], src[:], ACTF.Copy,
                             bias=0.0, scale=sc)

            if i + lookahead < len(sizes):
                emit_load(i + lookahead)
            SP.dma_start(y[:, off * OBITS:(off + tsz) * OBITS], yt[:])
    nc.compile()
    return nc


_NC_CACHE: dict = {}


def _get_nc(*_args) -> bass.Bass:
    if "nc" not in _NC_CACHE:
        _NC_CACHE["nc"] = build_nc()
    return _NC_CACHE["nc"]


def kernel(fp16_pulse: np.ndarray) -> np.ndarray:
    assert fp16_pulse.shape == (B0, B1, NBITS)
    in_dtype = fp16_pulse.dtype
    arr = np.ascontiguousarray(fp16_pulse, dtype=np.float16)
    in_maps = [
        {"x": arr[c * ROWS:(c + 1) * ROWS].reshape(P, VALS * NBITS)}
        for c in range(N_CORES)
    ]
    nc = _get_nc()
    res = run_bass_kernel_spmd(nc, in_maps, list(range(N_CORES)))
    out = np.empty((B0, B1, OBITS), dtype=np.float32)
    for c in range(N_CORES):
        out[c * ROWS:(c + 1) * ROWS] = (
            res.results[c]["y"].reshape(ROWS, B1, OBITS).astype(np.float32)
        )
    return out.astype(in_dtype, copy=False)


# revision 5
# speedup vs baseline: 1.0169x; 1.0169x over previous
"""FP16-pulse -> FP8(E4M3)-pulse converter, Trainium2 Bass/Tile kernel, v4.

v4 replaces v3's arithmetic rounding pipeline with the DVE's native
f16 -> f8e4 convert (verified exact vs the reference for every magnitude
pattern em <= 23551: RNE, subnormal outputs, and carry promote all match):

  em  = |fp16| bit pattern, assembled from the 15 magnitude pulse bits
        (7-lane pair op + base-4 tree, f32 lanes above 2048)
  em' = min(em, 23551)           # e>=23 handled by the +6 post-fix
  vham: Act writes em' to int16, bitcast f16 = |v| (exact)
  f8  = copy(|v| -> float8e4)    # the whole RNE/subnormal/saturate logic
  cb  = copy(bitcast u8 -> f16)  # 7-bit output code E*8+M
  c2  = cb + 6*(em > 23551)      # reference saturates every e>=23 to
                                 # (E,M)=(15,6); clamped input gives 120
  7 bits of c2 by is_ge/add chains; Act rescales each plane to 0/1 into
  the strided output planes; sign plane is a copy of input bit 0.

Transport: pulses cross HBM as float16 both ways (exact for 0/1; host only
casts dtype and slices -- all bit-level compute is on device).

Engine split (ISA-checked: Pool cannot run scalar_tensor_tensor): DVE runs
the assembly STTs and the first four extraction levels; Pool runs the em
clamp/overflow tensor_scalars, both convert copies and the last two
extraction levels; Act runs the int16 writeback and all eight output
planes. Loads and stores ride the SP HWDGE queue with 3-tile lookahead;
640-value mid tiles with (256,384) head and (512,384,128) tail tiles
shape fill and drain; the last two (small) tiles run their entire chain
on DVE alone (clamp, i16 writeback, converts, extraction, plane writes)
so the drain has no cross-engine waits. TimelineSim: 343.4us/core vs the
279.6us f16-transport DMA floor; DVE is 100% dense mid-stream (the
binding engine), so further gains need fewer DVE ops, not scheduling.
"""

import numpy as np
from contextlib import ExitStack

import concourse.bass as bass
import concourse.bacc as bacc
import concourse.tile as tile
from concourse import mybir
from concourse.bass_utils import run_bass_kernel_spmd

F32 = mybir.dt.float32
F16 = mybir.dt.float16
I16 = mybir.dt.int16
F8 = mybir.dt.float8e4
U8 = mybir.dt.uint8
OP = mybir.AluOpType
ACTF = mybir.ActivationFunctionType

P = 128
N_CORES = 8
B0, B1 = 4096, 4096
NBITS, OBITS = 16, 8

ROWS = B0 // N_CORES                    # 512 rows per core
VALS = ROWS * B1 // P                   # 16384 values per partition

# kept for test.py compatibility
VPT_FULL = 512
NTILES_FULL = VALS // VPT_FULL


def tile_sizes(mid=640, head=(256, 256), tail=(512, 384, 256)):
    head, tail = list(head), list(tail)
    n = (VALS - sum(head) - sum(tail)) // mid
    assert sum(head) + sum(tail) + n * mid == VALS
    return head + [mid] * n + tail


def build_nc(lookahead: int = 3, xbufs: int = 3, mid: int = 640,
             ext_split: int = 1, clamp_on_pool: bool = True,
             ext_plan: str | None = "VVVVGG", f8_on_pool: bool = True,
             cb_on_pool: bool = True, c2_on_pool: bool = False,
             chain_bufs: int = 2, head=(256, 384),
             tail=(512, 384, 128), act_tail_loads: int = 0,
             act_tail_stores: int = 0, fe_plan: str | None = None,
             dve_tail: int = 2) -> bass.Bass:
    # ext_plan: 6 chars from {'V','G','M'} per level: V=both DVE, G=both
    # Pool, M=mixed (TS on DVE, TT on Pool). Overrides ext_split.
    nc = bacc.Bacc()
    x = nc.declare_dram_parameter("x", [P, VALS * NBITS], F16, isOutput=False)
    y = nc.declare_dram_parameter("y", [P, VALS * OBITS], F16, isOutput=True)

    sizes = tile_sizes(mid=mid, head=head, tail=tail)

    with tile.TileContext(nc) as tc, ExitStack() as ctx:
        iop = ctx.enter_context(tc.tile_pool(name="io", bufs=2))
        tp = ctx.enter_context(tc.tile_pool(name="tmp", bufs=2))
        V, G, S, SP = nc.vector, nc.gpsimd, nc.scalar, nc.sync

        offs = np.cumsum([0] + sizes).tolist()
        xts = {}

        emitted = set()

        def emit_load(i):
            if i in emitted:
                return
            emitted.add(i)
            t = sizes[i]
            xt = iop.tile([P, NBITS * t], F16, tag="x", name="xt", bufs=xbufs)
            eng = S if i >= len(sizes) - act_tail_loads else SP
            eng.dma_start(xt[:], x[:, offs[i] * NBITS:(offs[i] + t) * NBITS])
            xts[i] = xt

        for k in range(min(lookahead, len(sizes))):
            emit_load(k)
        for i, tsz in enumerate(sizes):
            solo = i >= len(sizes) - dve_tail   # all-DVE drain tile
            xt = xts.pop(i)
            off = offs[i]
            xb = xt[:].rearrange("p (v b) -> p v b", b=NBITS)
            yt = iop.tile([P, OBITS * tsz], F16, tag="y", name="yt", bufs=3)
            yb = yt[:].rearrange("p (v b) -> p v b", b=OBITS)

            def vt(tag, dt=F16, w=1, bufs=None):
                if bufs:
                    return tp.tile([P, tsz * w], dt, tag=tag, name=tag,
                                   bufs=bufs)
                return tp.tile([P, tsz * w], dt, tag=tag, name=tag)

            # ---- DVE: em assembly (15 magnitude bits -> |v| pattern) ---
            # L1: 7 bit pairs p_j = 2*b(1+2j) + b(2+2j), j=0..6 (bits 1..14)
            pc = vt("pc", w=7)
            pcb = pc[:].rearrange("p (v k) -> p v k", k=7)
            V.scalar_tensor_tensor(pcb[:, :, :], xb[:, :, 1:14:2], 2.0,
                                   xb[:, :, 2:15:2], OP.mult, OP.add)
            # L2: q_j = 4*p(2j) + p(2j+1), j=0..2 (pairs of pairs)
            q3 = vt("q3", w=3)
            q3b = q3[:].rearrange("p (v k) -> p v k", k=3)
            V.scalar_tensor_tensor(q3b[:, :, :], pcb[:, :, 0:5:2], 4.0,
                                   pcb[:, :, 1:6:2], OP.mult, OP.add)
            # L3/L4: base-16 Horner over q0..q2, then p6 and b15
            r0 = vt("r0")                       # <= 255, f16 exact
            V.scalar_tensor_tensor(r0[:], q3b[:, :, 0], 16.0, q3b[:, :, 1],
                                   OP.mult, OP.add)
            r1 = vt("r1", F32)                  # <= 4095
            V.scalar_tensor_tensor(r1[:], r0[:], 16.0, q3b[:, :, 2],
                                   OP.mult, OP.add)
            H = vt("H", F32)                    # <= 16383
            V.scalar_tensor_tensor(H[:], r1[:], 4.0, pcb[:, :, 6],
                                   OP.mult, OP.add)
            em = vt("em", F32)                  # <= 32767
            V.scalar_tensor_tensor(em[:], H[:], 2.0, xb[:, :, 15],
                                   OP.mult, OP.add)

            # ---- clamp + overflow fix-up -------------------------------
            ceng = V if solo else (G if clamp_on_pool else V)
            emc = vt("emc", F32, bufs=chain_bufs)
            ceng.tensor_scalar(emc[:], em[:], 23551.0, None, OP.min)
            ovf6 = vt("ovf6", bufs=chain_bufs)
            ceng.tensor_scalar(ovf6[:], em[:], 23551.5, 6.0, OP.is_gt, OP.mult)

            # ---- Act int16 writeback, DVE f8 convert -------------------
            vham = vt("vham", I16, bufs=chain_bufs)
            if solo:
                V.tensor_scalar(vham[:], emc[:], 1.0, None, OP.mult)
            else:
                S.activation(vham[:], emc[:], ACTF.Copy, bias=0.0, scale=1.0)
            f8 = vt("f8", F8, bufs=chain_bufs)
            (V if solo else (G if f8_on_pool else V)).tensor_copy(
                f8[:], vham[:].bitcast(F16))
            cb = vt("cb", bufs=chain_bufs)
            (V if solo else (G if cb_on_pool else V)).tensor_copy(
                cb[:], f8[:].bitcast(U8))
            c2 = vt("c2", bufs=chain_bufs)
            (V if solo else (G if c2_on_pool else V)).tensor_tensor(
                c2[:], cb[:], ovf6[:], OP.add)

            # ---- bit extraction ----------------------------------------
            srcs = []   # (plane_idx, tile, scale)
            if fe_plan:
                # fE = floor(c2/8) via Act int16 RNE writeback (ties never
                # occur: fractions are j/8 - 0.4375). E and M extract in
                # two short parallel chains; fe_plan gives 5 level engines
                # (3 E-levels then 2 M-levels).
                fE = vt("fE", I16)
                S.activation(fE[:], c2[:], ACTF.Copy, bias=-0.4375,
                             scale=0.125)
                M = vt("M")
                V.scalar_tensor_tensor(M[:], fE[:], -8.0, c2[:],
                                       OP.mult, OP.add)
                chains = [(fE, [(1, 8), (2, 4), (3, 2)]),   # E3 E2 E1 (+E0)
                          (M, [(5, 4), (6, 2)])]            # M2 M1 (+M0)
                li = 0
                for r, levels in chains:
                    for pj, k in levels:
                        eng = V if fe_plan[li] == "V" else G
                        li += 1
                        bs = vt(f"fb{pj}")
                        eng.tensor_scalar(bs[:], r[:], float(k), -float(k),
                                          OP.is_ge, OP.mult)
                        rn = vt(f"fr{pj}")
                        eng.tensor_tensor(rn[:], r[:], bs[:], OP.add)
                        srcs.append((pj, bs, -1.0 / k))
                        r = rn
                    srcs.append((4 if levels[0][0] == 1 else 7, r, 1.0))
            else:
                r = c2
                for lvl in range(6):
                    k = 6 - lvl          # bit index being extracted
                    if solo:
                        ts_eng = tt_eng = V
                    elif ext_plan:
                        ts_eng = V if ext_plan[lvl] in "VM" else G
                        tt_eng = V if ext_plan[lvl] == "V" else G
                    else:
                        ts_eng = tt_eng = V if lvl < ext_split else G
                    bs = vt(f"b{k}s")
                    ts_eng.tensor_scalar(bs[:], r[:], float(1 << k),
                                         -float(1 << k), OP.is_ge, OP.mult)
                    rn = vt(f"rr{k - 1}")
                    tt_eng.tensor_tensor(rn[:], r[:], bs[:], OP.add)
                    srcs.append((7 - k, bs, -1.0 / (1 << k)))
                    r = rn
                srcs.append((7, r, 1.0))  # bit 0, already 0/1

            # ---- output planes (Act; all-DVE for drain tiles) ----------
            if solo:
                V.tensor_scalar(yb[:, :, 0], xb[:, :, 0], 1.0, None, OP.mult)
                for pj, src, sc in srcs:
                    V.tensor_scalar(yb[:, :, pj], src[:], sc, None, OP.mult)
            else:
                S.activation(yb[:, :, 0], xb[:, :, 0], ACTF.Copy,
                             bias=0.0, scale=1.0)
                for pj, src, sc in srcs:
                    S.activation(yb[:, :, pj], src[:], ACTF.Copy,
                                 bias=0.0, scale=sc)

            nxt = i + lookahead
            if nxt < len(sizes):
                emit_load(nxt)
                # bunch the final loads so no store separates them in the
                # SP queue (endgame stores issue at compute pace and would
                # delay the last loads, starving DVE in the drain)
                while nxt == len(sizes) - 2:
                    nxt += 1
                    emit_load(nxt)
            seng = S if i >= len(sizes) - act_tail_stores else SP
            seng.dma_start(y[:, off * OBITS:(off + tsz) * OBITS], yt[:])
    nc.compile()
    return nc


_NC_CACHE: dict = {}


def _get_nc(*_args) -> bass.Bass:
    if "nc" not in _NC_CACHE:
        _NC_CACHE["nc"] = build_nc()
    return _NC_CACHE["nc"]


def kernel(fp16_pulse: np.ndarray) -> np.ndarray:
    assert fp16_pulse.shape == (B0, B1, NBITS)
    in_dtype = fp16_pulse.dtype
    arr = np.ascontiguousarray(fp16_pulse, dtype=np.float16)
    in_maps = [
        {"x": arr[c * ROWS:(c + 1) * ROWS].reshape(P, VALS * NBITS)}
        for c in range(N_CORES)
    ]
    nc = _get_nc()
    res = run_bass_kernel_spmd(nc, in_maps, list(range(N_CORES)))
    out = np.empty((B0, B1, OBITS), dtype=np.float32)
    for c in range(N_CORES):
        out[c * ROWS:(c + 1) * ROWS] = (
            res.results[c]["y"].reshape(ROWS, B1, OBITS).astype(np.float32)
        )
    return out.astype(in_dtype, copy=False)


# revision 6
# speedup vs baseline: 1.0207x; 1.0038x over previous
"""FP16-pulse -> FP8(E4M3)-pulse converter, Trainium2 Bass/Tile kernel, v4.

v4 replaces v3's arithmetic rounding pipeline with the DVE's native
f16 -> f8e4 convert (verified exact vs the reference for every magnitude
pattern em <= 23551: RNE, subnormal outputs, and carry promote all match):

  em  = |fp16| bit pattern, assembled from the 15 magnitude pulse bits
        (7-lane pair op + base-4 tree, f32 lanes above 2048)
  em' = min(em, 23551)           # e>=23 handled by the +6 post-fix
  vham: Act writes em' to int16, bitcast f16 = |v| (exact)
  f8  = copy(|v| -> float8e4)    # the whole RNE/subnormal/saturate logic
  cb  = copy(bitcast u8 -> f16)  # 7-bit output code E*8+M
  c2  = cb + 6*(em > 23551)      # reference saturates every e>=23 to
                                 # (E,M)=(15,6); clamped input gives 120
  7 bits of c2 by is_ge/add chains; Act rescales each plane to 0/1 into
  the strided output planes; sign plane is a copy of input bit 0.

Transport: pulses cross HBM as float16 both ways (exact for 0/1; host only
casts dtype and slices -- all bit-level compute is on device).

Engine split (ISA-checked: Pool cannot run scalar_tensor_tensor): DVE runs
the assembly STTs and the first four extraction levels; Pool runs the em
clamp/overflow tensor_scalars, both convert copies and the last two
extraction levels; Act runs the int16 writeback and all eight output
planes. Loads and stores ride the SP HWDGE queue with 3-tile lookahead;
640-value mid tiles with (256,384) head and (640,256,128) tail tiles
shape fill and drain; the last two (small) tiles run their entire chain
on DVE alone (clamp, i16 writeback, converts, extraction, plane writes)
so the drain has no cross-engine waits. TimelineSim: 342.1us/core vs the
279.6us f16-transport DMA floor; DVE is 100% dense mid-stream (the
binding engine), so further gains need fewer DVE ops, not scheduling.
"""

import numpy as np
from contextlib import ExitStack

import concourse.bass as bass
import concourse.bacc as bacc
import concourse.tile as tile
from concourse import mybir
from concourse.bass_utils import run_bass_kernel_spmd

F32 = mybir.dt.float32
F16 = mybir.dt.float16
I16 = mybir.dt.int16
F8 = mybir.dt.float8e4
U8 = mybir.dt.uint8
OP = mybir.AluOpType
ACTF = mybir.ActivationFunctionType

P = 128
N_CORES = 8
B0, B1 = 4096, 4096
NBITS, OBITS = 16, 8

ROWS = B0 // N_CORES                    # 512 rows per core
VALS = ROWS * B1 // P                   # 16384 values per partition

# kept for test.py compatibility
VPT_FULL = 512
NTILES_FULL = VALS // VPT_FULL


def tile_sizes(mid=640, head=(256, 256), tail=(512, 384, 256)):
    head, tail = list(head), list(tail)
    n = (VALS - sum(head) - sum(tail)) // mid
    assert sum(head) + sum(tail) + n * mid == VALS
    return head + [mid] * n + tail


def build_nc(lookahead: int = 3, xbufs: int = 3, mid: int = 640,
             ext_split: int = 1, clamp_on_pool: bool = True,
             ext_plan: str | None = "VVVVGG", f8_on_pool: bool = True,
             cb_on_pool: bool = True, c2_on_pool: bool = False,
             chain_bufs: int = 2, head=(256, 384),
             tail=(640, 256, 128), act_tail_loads: int = 0,
             act_tail_stores: int = 0, fe_plan: str | None = None,
             dve_tail: int = 2) -> bass.Bass:
    # ext_plan: 6 chars from {'V','G','M'} per level: V=both DVE, G=both
    # Pool, M=mixed (TS on DVE, TT on Pool). Overrides ext_split.
    nc = bacc.Bacc()
    x = nc.declare_dram_parameter("x", [P, VALS * NBITS], F16, isOutput=False)
    y = nc.declare_dram_parameter("y", [P, VALS * OBITS], F16, isOutput=True)

    sizes = tile_sizes(mid=mid, head=head, tail=tail)

    with tile.TileContext(nc) as tc, ExitStack() as ctx:
        iop = ctx.enter_context(tc.tile_pool(name="io", bufs=2))
        tp = ctx.enter_context(tc.tile_pool(name="tmp", bufs=2))
        V, G, S, SP = nc.vector, nc.gpsimd, nc.scalar, nc.sync

        offs = np.cumsum([0] + sizes).tolist()
        xts = {}

        emitted = set()

        def emit_load(i):
            if i in emitted:
                return
            emitted.add(i)
            t = sizes[i]
            xt = iop.tile([P, NBITS * t], F16, tag="x", name="xt", bufs=xbufs)
            eng = S if i >= len(sizes) - act_tail_loads else SP
            eng.dma_start(xt[:], x[:, offs[i] * NBITS:(offs[i] + t) * NBITS])
            xts[i] = xt

        for k in range(min(lookahead, len(sizes))):
            emit_load(k)
        for i, tsz in enumerate(sizes):
            solo = i >= len(sizes) - dve_tail   # all-DVE drain tile
            xt = xts.pop(i)
            off = offs[i]
            xb = xt[:].rearrange("p (v b) -> p v b", b=NBITS)
            yt = iop.tile([P, OBITS * tsz], F16, tag="y", name="yt", bufs=3)
            yb = yt[:].rearrange("p (v b) -> p v b", b=OBITS)

            def vt(tag, dt=F16, w=1, bufs=None):
                if bufs:
                    return tp.tile([P, tsz * w], dt, tag=tag, name=tag,
                                   bufs=bufs)
                return tp.tile([P, tsz * w], dt, tag=tag, name=tag)

            # ---- DVE: em assembly (15 magnitude bits -> |v| pattern) ---
            # L1: 7 bit pairs p_j = 2*b(1+2j) + b(2+2j), j=0..6 (bits 1..14)
            pc = vt("pc", w=7)
            pcb = pc[:].rearrange("p (v k) -> p v k", k=7)
            V.scalar_tensor_tensor(pcb[:, :, :], xb[:, :, 1:14:2], 2.0,
                                   xb[:, :, 2:15:2], OP.mult, OP.add)
            # L2: q_j = 4*p(2j) + p(2j+1), j=0..2 (pairs of pairs)
            q3 = vt("q3", w=3)
            q3b = q3[:].rearrange("p (v k) -> p v k", k=3)
            V.scalar_tensor_tensor(q3b[:, :, :], pcb[:, :, 0:5:2], 4.0,
                                   pcb[:, :, 1:6:2], OP.mult, OP.add)
            # L3/L4: base-16 Horner over q0..q2, then p6 and b15
            r0 = vt("r0")                       # <= 255, f16 exact
            V.scalar_tensor_tensor(r0[:], q3b[:, :, 0], 16.0, q3b[:, :, 1],
                                   OP.mult, OP.add)
            r1 = vt("r1", F32)                  # <= 4095
            V.scalar_tensor_tensor(r1[:], r0[:], 16.0, q3b[:, :, 2],
                                   OP.mult, OP.add)
            H = vt("H", F32)                    # <= 16383
            V.scalar_tensor_tensor(H[:], r1[:], 4.0, pcb[:, :, 6],
                                   OP.mult, OP.add)
            em = vt("em", F32)                  # <= 32767
            V.scalar_tensor_tensor(em[:], H[:], 2.0, xb[:, :, 15],
                                   OP.mult, OP.add)

            # ---- clamp + overflow fix-up -------------------------------
            ceng = V if solo else (G if clamp_on_pool else V)
            emc = vt("emc", F32, bufs=chain_bufs)
            ceng.tensor_scalar(emc[:], em[:], 23551.0, None, OP.min)
            ovf6 = vt("ovf6", bufs=chain_bufs)
            ceng.tensor_scalar(ovf6[:], em[:], 23551.5, 6.0, OP.is_gt, OP.mult)

            # ---- Act int16 writeback, DVE f8 convert -------------------
            vham = vt("vham", I16, bufs=chain_bufs)
            if solo:
                V.tensor_scalar(vham[:], emc[:], 1.0, None, OP.mult)
            else:
                S.activation(vham[:], emc[:], ACTF.Copy, bias=0.0, scale=1.0)
            f8 = vt("f8", F8, bufs=chain_bufs)
            (V if solo else (G if f8_on_pool else V)).tensor_copy(
                f8[:], vham[:].bitcast(F16))
            cb = vt("cb", bufs=chain_bufs)
            (V if solo else (G if cb_on_pool else V)).tensor_copy(
                cb[:], f8[:].bitcast(U8))
            c2 = vt("c2", bufs=chain_bufs)
            (V if solo else (G if c2_on_pool else V)).tensor_tensor(
                c2[:], cb[:], ovf6[:], OP.add)

            # ---- bit extraction ----------------------------------------
            srcs = []   # (plane_idx, tile, scale)
            if fe_plan:
                # fE = floor(c2/8) via Act int16 RNE writeback (ties never
                # occur: fractions are j/8 - 0.4375). E and M extract in
                # two short parallel chains; fe_plan gives 5 level engines
                # (3 E-levels then 2 M-levels).
                fE = vt("fE", I16)
                S.activation(fE[:], c2[:], ACTF.Copy, bias=-0.4375,
                             scale=0.125)
                M = vt("M")
                V.scalar_tensor_tensor(M[:], fE[:], -8.0, c2[:],
                                       OP.mult, OP.add)
                chains = [(fE, [(1, 8), (2, 4), (3, 2)]),   # E3 E2 E1 (+E0)
                          (M, [(5, 4), (6, 2)])]            # M2 M1 (+M0)
                li = 0
                for r, levels in chains:
                    for pj, k in levels:
                        eng = V if fe_plan[li] == "V" else G
                        li += 1
                        bs = vt(f"fb{pj}")
                        eng.tensor_scalar(bs[:], r[:], float(k), -float(k),
                                          OP.is_ge, OP.mult)
                        rn = vt(f"fr{pj}")
                        eng.tensor_tensor(rn[:], r[:], bs[:], OP.add)
                        srcs.append((pj, bs, -1.0 / k))
                        r = rn
                    srcs.append((4 if levels[0][0] == 1 else 7, r, 1.0))
            else:
                r = c2
                for lvl in range(6):
                    k = 6 - lvl          # bit index being extracted
                    if solo:
                        ts_eng = tt_eng = V
                    elif ext_plan:
                        ts_eng = V if ext_plan[lvl] in "VM" else G
                        tt_eng = V if ext_plan[lvl] == "V" else G
                    else:
                        ts_eng = tt_eng = V if lvl < ext_split else G
                    bs = vt(f"b{k}s")
                    ts_eng.tensor_scalar(bs[:], r[:], float(1 << k),
                                         -float(1 << k), OP.is_ge, OP.mult)
                    rn = vt(f"rr{k - 1}")
                    tt_eng.tensor_tensor(rn[:], r[:], bs[:], OP.add)
                    srcs.append((7 - k, bs, -1.0 / (1 << k)))
                    r = rn
                srcs.append((7, r, 1.0))  # bit 0, already 0/1

            # ---- output planes (Act; all-DVE for drain tiles) ----------
            if solo:
                V.tensor_scalar(yb[:, :, 0], xb[:, :, 0], 1.0, None, OP.mult)
                for pj, src, sc in srcs:
                    V.tensor_scalar(yb[:, :, pj], src[:], sc, None, OP.mult)
            else:
                S.activation(yb[:, :, 0], xb[:, :, 0], ACTF.Copy,
                             bias=0.0, scale=1.0)
                for pj, src, sc in srcs:
                    S.activation(yb[:, :, pj], src[:], ACTF.Copy,
                                 bias=0.0, scale=sc)

            nxt = i + lookahead
            if nxt < len(sizes):
                emit_load(nxt)
                # bunch the final loads so no store separates them in the
                # SP queue (endgame stores issue at compute pace and would
                # delay the last loads, starving DVE in the drain)
                while nxt == len(sizes) - 2:
                    nxt += 1
                    emit_load(nxt)
            seng = S if i >= len(sizes) - act_tail_stores else SP
            seng.dma_start(y[:, off * OBITS:(off + tsz) * OBITS], yt[:])
    nc.compile()
    return nc


_NC_CACHE: dict = {}


def _get_nc(*_args) -> bass.Bass:
    if "nc" not in _NC_CACHE:
        _NC_CACHE["nc"] = build_nc()
    return _NC_CACHE["nc"]


def kernel(fp16_pulse: np.ndarray) -> np.ndarray:
    assert fp16_pulse.shape == (B0, B1, NBITS)
    in_dtype = fp16_pulse.dtype
    arr = np.ascontiguousarray(fp16_pulse, dtype=np.float16)
    in_maps = [
        {"x": arr[c * ROWS:(c + 1) * ROWS].reshape(P, VALS * NBITS)}
        for c in range(N_CORES)
    ]
    nc = _get_nc()
    res = run_bass_kernel_spmd(nc, in_maps, list(range(N_CORES)))
    out = np.empty((B0, B1, OBITS), dtype=np.float32)
    for c in range(N_CORES):
        out[c * ROWS:(c + 1) * ROWS] = (
            res.results[c]["y"].reshape(ROWS, B1, OBITS).astype(np.float32)
        )
    return out.astype(in_dtype, copy=False)


# revision 7
# speedup vs baseline: 1.0358x; 1.0148x over previous
"""FP16-pulse -> FP8(E4M3)-pulse converter, Trainium2 Bass/Tile kernel, v4.

v4 replaces v3's arithmetic rounding pipeline with the DVE's native
f16 -> f8e4 convert (verified exact vs the reference for every magnitude
pattern em <= 23551: RNE, subnormal outputs, and carry promote all match):

  em  = |fp16| bit pattern, assembled from the 15 magnitude pulse bits
        (7-lane pair op + base-4 tree, f32 lanes above 2048)
  em' = min(em, 23551)           # e>=23 handled by the +6 post-fix
  vham: Act writes em' to int16, bitcast f16 = |v| (exact)
  f8  = copy(|v| -> float8e4)    # the whole RNE/subnormal/saturate logic
  cb  = copy(bitcast u8 -> f16)  # 7-bit output code E*8+M
  c2  = cb + 6*(em > 23551)      # reference saturates every e>=23 to
                                 # (E,M)=(15,6); clamped input gives 120
  7 bits of c2 by is_ge/add chains; Act rescales each plane to 0/1 into
  the strided output planes; sign plane is a copy of input bit 0.

Transport: pulses cross HBM as float16 both ways (exact for 0/1; host only
casts dtype and slices -- all bit-level compute is on device).

Engine split (ISA-checked: Pool cannot run scalar_tensor_tensor): DVE runs
the assembly STTs and the first four extraction levels; Pool runs the em
clamp/overflow tensor_scalars, both convert copies and the last two
extraction levels; Act runs the int16 writeback and all eight output
planes. Loads and stores ride the SP HWDGE queue with 3-tile lookahead;
640-value mid tiles with (256,384) head and (640,256,128) tail tiles
shape fill and drain; the last two (small) tiles run their entire chain
on DVE alone (clamp, i16 writeback, converts, extraction, plane writes)
so the drain has no cross-engine waits. TimelineSim: 342.1us/core vs the
279.6us f16-transport DMA floor; DVE is 100% dense mid-stream (the
binding engine), so further gains need fewer DVE ops, not scheduling.
"""

import numpy as np
from contextlib import ExitStack

import concourse.bass as bass
import concourse.bacc as bacc
import concourse.tile as tile
from concourse import mybir
from concourse.bass_utils import run_bass_kernel_spmd

F32 = mybir.dt.float32
F16 = mybir.dt.float16
I16 = mybir.dt.int16
F8 = mybir.dt.float8e4
U8 = mybir.dt.uint8
OP = mybir.AluOpType
ACTF = mybir.ActivationFunctionType

P = 128
N_CORES = 8
B0, B1 = 4096, 4096
NBITS, OBITS = 16, 8

ROWS = B0 // N_CORES                    # 512 rows per core
VALS = ROWS * B1 // P                   # 16384 values per partition

# kept for test.py compatibility
VPT_FULL = 512
NTILES_FULL = VALS // VPT_FULL


def tile_sizes(mid=640, head=(256, 256), tail=(512, 384, 256)):
    head, tail = list(head), list(tail)
    n = (VALS - sum(head) - sum(tail)) // mid
    assert sum(head) + sum(tail) + n * mid == VALS
    return head + [mid] * n + tail


def build_nc(lookahead: int = 3, xbufs: int = 3, mid: int = 640,
             ext_split: int = 1, clamp_on_pool: bool = True,
             ext_plan: str | None = "VVVVGG", f8_on_pool: bool = True,
             cb_on_pool: bool = True, c2_on_pool: bool = False,
             chain_bufs: int = 2, head=(256, 384),
             tail=(640, 256, 128), act_tail_loads: int = 0,
             act_tail_stores: int = 0, fe_plan: str | None = None,
             dve_tail: int = 2) -> bass.Bass:
    # ext_plan: 6 chars from {'V','G','M'} per level: V=both DVE, G=both
    # Pool, M=mixed (TS on DVE, TT on Pool). Overrides ext_split.
    nc = bacc.Bacc()
    x = nc.declare_dram_parameter("x", [P, VALS * NBITS], F16, isOutput=False)
    y = nc.declare_dram_parameter("y", [P, VALS * OBITS], F16, isOutput=True)

    sizes = tile_sizes(mid=mid, head=head, tail=tail)

    with tile.TileContext(nc) as tc, ExitStack() as ctx:
        iop = ctx.enter_context(tc.tile_pool(name="io", bufs=2))
        tp = ctx.enter_context(tc.tile_pool(name="tmp", bufs=2))
        V, G, S, SP = nc.vector, nc.gpsimd, nc.scalar, nc.sync

        offs = np.cumsum([0] + sizes).tolist()
        xts = {}

        emitted = set()

        def emit_load(i):
            if i in emitted:
                return
            emitted.add(i)
            t = sizes[i]
            xt = iop.tile([P, NBITS * t], F16, tag="x", name="xt", bufs=xbufs)
            eng = S if i >= len(sizes) - act_tail_loads else SP
            eng.dma_start(xt[:], x[:, offs[i] * NBITS:(offs[i] + t) * NBITS])
            xts[i] = xt

        for k in range(min(lookahead, len(sizes))):
            emit_load(k)
        for i, tsz in enumerate(sizes):
            solo = i >= len(sizes) - dve_tail   # all-DVE drain tile
            xt = xts.pop(i)
            off = offs[i]
            xb = xt[:].rearrange("p (v b) -> p v b", b=NBITS)
            yt = iop.tile([P, OBITS * tsz], F16, tag="y", name="yt", bufs=3)
            yb = yt[:].rearrange("p (v b) -> p v b", b=OBITS)

            def vt(tag, dt=F16, w=1, bufs=None):
                if bufs:
                    return tp.tile([P, tsz * w], dt, tag=tag, name=tag,
                                   bufs=bufs)
                return tp.tile([P, tsz * w], dt, tag=tag, name=tag)

            # ---- DVE: em assembly (15 magnitude bits -> |v| pattern) ---
            # L1: 7 bit pairs p_j = 2*b(1+2j) + b(2+2j), j=0..6 (bits 1..14)
            pc = vt("pc", w=7)
            pcb = pc[:].rearrange("p (v k) -> p v k", k=7)
            V.scalar_tensor_tensor(pcb[:, :, :], xb[:, :, 1:14:2], 2.0,
                                   xb[:, :, 2:15:2], OP.mult, OP.add)
            # L2: q_j = 4*p(2j) + p(2j+1), j=0..2 (pairs of pairs)
            q3 = vt("q3", w=3)
            q3b = q3[:].rearrange("p (v k) -> p v k", k=3)
            V.scalar_tensor_tensor(q3b[:, :, :], pcb[:, :, 0:5:2], 4.0,
                                   pcb[:, :, 1:6:2], OP.mult, OP.add)
            # L3/L4: base-16 Horner over q0..q2, then p6 and b15
            r0 = vt("r0")                       # <= 255, f16 exact
            V.scalar_tensor_tensor(r0[:], q3b[:, :, 0], 16.0, q3b[:, :, 1],
                                   OP.mult, OP.add)
            r1 = vt("r1", F32)                  # <= 4095
            V.scalar_tensor_tensor(r1[:], r0[:], 16.0, q3b[:, :, 2],
                                   OP.mult, OP.add)
            H = vt("H", F32)                    # <= 16383
            V.scalar_tensor_tensor(H[:], r1[:], 4.0, pcb[:, :, 6],
                                   OP.mult, OP.add)
            em = vt("em", F32)                  # <= 32767
            V.scalar_tensor_tensor(em[:], H[:], 2.0, xb[:, :, 15],
                                   OP.mult, OP.add)

            # ---- clamp + overflow fix-up -------------------------------
            ceng = V if solo else (G if clamp_on_pool else V)
            emc = vt("emc", F32, bufs=chain_bufs)
            ceng.tensor_scalar(emc[:], em[:], 23551.0, None, OP.min)
            ovf6 = vt("ovf6", bufs=chain_bufs)
            ceng.tensor_scalar(ovf6[:], em[:], 23551.5, 6.0, OP.is_gt, OP.mult)

            # bit6 (E>=8) is monotone in em: it equals (em >= 16320), the
            # RNE boundary where |v| rounds up to 2.0. Computing its -64
            # form from em on Pool takes the level-0 compare off DVE.
            b6e = vt("b6e")
            (V if solo else G).tensor_scalar(b6e[:], em[:], 16319.5, -64.0,
                                             OP.is_ge, OP.mult)

            # ---- Act int16 writeback, DVE f8 convert -------------------
            vham = vt("vham", I16, bufs=chain_bufs)
            if solo:
                V.tensor_scalar(vham[:], emc[:], 1.0, None, OP.mult)
            else:
                S.activation(vham[:], emc[:], ACTF.Copy, bias=0.0, scale=1.0)
            f8 = vt("f8", F8, bufs=chain_bufs)
            (V if solo else (G if f8_on_pool else V)).tensor_copy(
                f8[:], vham[:].bitcast(F16))
            cb = vt("cb", bufs=chain_bufs)
            (V if solo else (G if cb_on_pool else V)).tensor_copy(
                cb[:], f8[:].bitcast(U8))
            c2 = vt("c2", bufs=chain_bufs)
            V.tensor_tensor(c2[:], cb[:], ovf6[:], OP.add)

            # ---- bit extraction ----------------------------------------
            srcs = []   # (plane_idx, tile, scale)
            if fe_plan:
                # fE = floor(c2/8) via Act int16 RNE writeback (ties never
                # occur: fractions are j/8 - 0.4375). E and M extract in
                # two short parallel chains; fe_plan gives 5 level engines
                # (3 E-levels then 2 M-levels).
                fE = vt("fE", I16)
                S.activation(fE[:], c2[:], ACTF.Copy, bias=-0.4375,
                             scale=0.125)
                M = vt("M")
                V.scalar_tensor_tensor(M[:], fE[:], -8.0, c2[:],
                                       OP.mult, OP.add)
                chains = [(fE, [(1, 8), (2, 4), (3, 2)]),   # E3 E2 E1 (+E0)
                          (M, [(5, 4), (6, 2)])]            # M2 M1 (+M0)
                li = 0
                for r, levels in chains:
                    for pj, k in levels:
                        eng = V if fe_plan[li] == "V" else G
                        li += 1
                        bs = vt(f"fb{pj}")
                        eng.tensor_scalar(bs[:], r[:], float(k), -float(k),
                                          OP.is_ge, OP.mult)
                        rn = vt(f"fr{pj}")
                        eng.tensor_tensor(rn[:], r[:], bs[:], OP.add)
                        srcs.append((pj, bs, -1.0 / k))
                        r = rn
                    srcs.append((4 if levels[0][0] == 1 else 7, r, 1.0))
            else:
                r = c2
                for lvl in range(6):
                    k = 6 - lvl          # bit index being extracted
                    if solo:
                        ts_eng = tt_eng = V
                    elif ext_plan:
                        ts_eng = V if ext_plan[lvl] in "VM" else G
                        tt_eng = V if ext_plan[lvl] == "V" else G
                    else:
                        ts_eng = tt_eng = V if lvl < ext_split else G
                    if lvl == 0:
                        bs = b6e
                    else:
                        bs = vt(f"b{k}s")
                        ts_eng.tensor_scalar(bs[:], r[:], float(1 << k),
                                             -float(1 << k), OP.is_ge,
                                             OP.mult)
                    rn = vt(f"rr{k - 1}")
                    tt_eng.tensor_tensor(rn[:], r[:], bs[:], OP.add)
                    srcs.append((7 - k, bs, -1.0 / (1 << k)))
                    r = rn
                srcs.append((7, r, 1.0))  # bit 0, already 0/1

            # ---- output planes (Act; all-DVE for drain tiles) ----------
            if solo:
                V.tensor_scalar(yb[:, :, 0], xb[:, :, 0], 1.0, None, OP.mult)
                for pj, src, sc in srcs:
                    V.tensor_scalar(yb[:, :, pj], src[:], sc, None, OP.mult)
            else:
                S.activation(yb[:, :, 0], xb[:, :, 0], ACTF.Copy,
                             bias=0.0, scale=1.0)
                for pj, src, sc in srcs:
                    S.activation(yb[:, :, pj], src[:], ACTF.Copy,
                                 bias=0.0, scale=sc)

            nxt = i + lookahead
            if nxt < len(sizes):
                emit_load(nxt)
                # bunch the final loads so no store separates them in the
                # SP queue (endgame stores issue at compute pace and would
                # delay the last loads, starving DVE in the drain)
                while nxt == len(sizes) - 2:
                    nxt += 1
                    emit_load(nxt)
            seng = S if i >= len(sizes) - act_tail_stores else SP
            seng.dma_start(y[:, off * OBITS:(off + tsz) * OBITS], yt[:])
    nc.compile()
    return nc


_NC_CACHE: dict = {}


def _get_nc(*_args) -> bass.Bass:
    if "nc" not in _NC_CACHE:
        _NC_CACHE["nc"] = build_nc()
    return _NC_CACHE["nc"]


def kernel(fp16_pulse: np.ndarray) -> np.ndarray:
    assert fp16_pulse.shape == (B0, B1, NBITS)
    in_dtype = fp16_pulse.dtype
    arr = np.ascontiguousarray(fp16_pulse, dtype=np.float16)
    in_maps = [
        {"x": arr[c * ROWS:(c + 1) * ROWS].reshape(P, VALS * NBITS)}
        for c in range(N_CORES)
    ]
    nc = _get_nc()
    res = run_bass_kernel_spmd(nc, in_maps, list(range(N_CORES)))
    out = np.empty((B0, B1, OBITS), dtype=np.float32)
    for c in range(N_CORES):
        out[c * ROWS:(c + 1) * ROWS] = (
            res.results[c]["y"].reshape(ROWS, B1, OBITS).astype(np.float32)
        )
    return out.astype(in_dtype, copy=False)
